# revision 9
# baseline (speedup 1.0000x reference)
"""Trainium2 Bass kernel v2 for DecoderAttention (B=16, T=1024, D=1024, H=16).

Data-parallel over batch: 2 items/core, no collectives.

Changes vs v1 baseline:
- All matmul operands bf16 (same PE rate as f32r at N>=512, but: transposes
  run 1.0 c/row instead of 1.5, SBUF footprint halves, DVE copies get 2x).
- w_qkv / w_out loaded + converted to bf16 ONCE, resident in SBUF across
  both batch items (no DMA inside the C-window).
- Softmax-denominator broadcast moved off PE: GpSimd partition_broadcast
  replaces the K=1 broadcast matmuls (saves ~16k PE cycles/item).
- hsT double-buffered across items so the PE stream never drains at item
  boundaries.
"""

import os
import sys

import numpy as np

sys.path.insert(0, "/opt/trn_rl_repo")

import concourse.bass as bass  # noqa: E402
import concourse.mybir as mybir  # noqa: E402
import concourse.tile as tile  # noqa: E402
from concourse import bacc  # noqa: E402
from concourse.bass_utils import run_bass_kernel_spmd  # noqa: E402
from concourse.masks import make_identity  # noqa: E402

F32 = mybir.dt.float32
BF16 = mybir.dt.bfloat16

B, T, D = 16, 1024, 1024
H, HD = 16, 64
N_CORES = 8
BL = B // N_CORES
P = 128
CT = D // P  # 8 contraction tiles
TT = T // P  # 8 token tiles
NQ = 512
SCALE = 1.0 / np.sqrt(HD)

_last_results = None


def build_program():
    nc = bacc.Bacc(
        "TRN2", target_bir_lowering=False, debug=False, num_devices=N_CORES
    )

    hs = nc.dram_tensor("hidden_states", [BL, T, D], F32, kind="ExternalInput")
    w_qkv = nc.dram_tensor("w_qkv", [D, 3 * D], F32, kind="ExternalInput")
    b_qkv = nc.dram_tensor("b_qkv", [3 * D], F32, kind="ExternalInput")
    w_out = nc.dram_tensor("w_out", [D, D], F32, kind="ExternalInput")
    b_out = nc.dram_tensor("b_out", [D], F32, kind="ExternalInput")
    out = nc.dram_tensor("out", [BL, T, D], F32, kind="ExternalOutput")

    Exp = mybir.ActivationFunctionType.Exp
    add = mybir.AluOpType.add
    mult = mybir.AluOpType.mult

    with tile.TileContext(nc) as tc:
        with (
            tc.tile_pool(name="consts", bufs=1) as consts,
            tc.tile_pool(name="wpool", bufs=1) as wpool,
            tc.tile_pool(name="main", bufs=1) as main,
            tc.tile_pool(name="pipe", bufs=2) as pipe,
            tc.tile_pool(name="psum", bufs=1, space="PSUM") as psum,
        ):
            # ---------------- constants ----------------
            identity_b = consts.tile([P, P], BF16)
            make_identity(nc, identity_b)
            ones_ph = consts.tile([P, H, 1], BF16)
            nc.gpsimd.memset(ones_ph, 1.0)
            bq = consts.tile([P, 2 * CT], F32)
            nc.sync.dma_start(
                out=bq, in_=b_qkv.rearrange("(i p) -> p i", p=P)[:, 0 : 2 * CT]
            )
            bv_row = pipe.tile([1, D], BF16, tag="brow", name="bv_row")
            bout_row = pipe.tile([1, D], BF16, tag="brow", name="bout_row")
            bv_row32 = pipe.tile([1, D], F32, tag="o", name="bv_row32")
            bout_row32 = pipe.tile([1, D], F32, tag="o", name="bout_row32")
            nc.sync.dma_start(out=bv_row32, in_=b_qkv[2 * D : 3 * D][None, :])
            with nc.allow_low_precision(reason="bf16 bias"):
                nc.vector.tensor_copy(bv_row, bv_row32)
            nc.sync.dma_start(out=bout_row32, in_=b_out[None, :])
            with nc.allow_low_precision(reason="bf16 bias"):
                nc.vector.tensor_copy(bout_row, bout_row32)
            bcast_bv = consts.tile([P, D], BF16)
            bcast_bout = consts.tile([P, D], BF16)
            nc.gpsimd.partition_broadcast(bcast_bv, bv_row)
            nc.gpsimd.partition_broadcast(bcast_bout, bout_row)

            # ---------------- resident weights (bf16) ----------------
            # w_all[p, c, j]: w_qkv row c*128+p, col j ; wout_sb[p, c, e]
            w_all = wpool.tile([P, CT, 3 * D], BF16)
            wout_sb = wpool.tile([P, CT, D], BF16)

            def emit_v_weight_loads():
                # V-slice on DVE (fast) so V-proj can start early
                for c in range(CT):
                    vst = pipe.tile([P, D], F32, tag="wst", bufs=2,
                                    name=f"vst{c}")
                    nc.sync.dma_start(
                        out=vst, in_=w_qkv[c * P : (c + 1) * P, 2 * D : 3 * D]
                    )
                    with nc.allow_low_precision(reason="bf16 weights"):
                        nc.vector.tensor_copy(w_all[:, c, 2 * D : 3 * D], vst)

            w_qkv_pcj = w_qkv.rearrange("(c p) j -> p c j", p=P)

            def emit_qk_slice_load(jt):
                # stage + convert one 128-col j-slice of w_qkv across all 8
                # c-tiles; emitted mid-C-window where Pool/DMA are idle
                stg = pipe.tile([P, CT, P], F32, tag="wqs", bufs=2,
                                name=f"wqs{jt}")
                nc.sync.dma_start(
                    out=stg, in_=w_qkv_pcj[:, :, jt * P : (jt + 1) * P]
                )
                with nc.allow_low_precision(reason="bf16 weights"):
                    nc.gpsimd.tensor_copy(
                        w_all[:, :, jt * P : (jt + 1) * P], stg
                    )

            def emit_wout_load(dt):
                wost = pipe.tile([P, CT, P], F32, tag="wqs", bufs=2,
                                 name=f"wost{dt}")
                wosl = wost.rearrange("p c j -> p (c j)")
                nc.sync.dma_start(
                    out=wosl, in_=w_out[dt * P : (dt + 1) * P, :]
                )
                with nc.allow_low_precision(reason="bf16 weights"):
                    nc.gpsimd.tensor_copy(wout_sb[:, dt, :], wosl)

            emit_v_weight_loads()

            # double-buffered across items
            hsT = [
                main.tile([P, CT, T], BF16, tag=f"hsT{par}", name=f"hsT{par}")
                for par in range(2)
            ]

            def emit_hs_prefetch(b, thalf):
                # DMA + Pool f32->bf16 convert for one t-half; emitted early
                # (during the previous item's out-projection) so the PE never
                # waits on hs at the item boundary
                hsb = []
                for i in range(4):
                    t = thalf * 4 + i
                    st = pipe.tile([P, D], F32, tag="hst", bufs=2,
                                   name=f"hst{b}_{t}")
                    eng = nc.sync if i % 2 == 0 else nc.gpsimd
                    eng.dma_start(out=st, in_=hs[b, t * P : (t + 1) * P, :])
                    hb = pipe.tile([P, D], BF16, tag="hsb", bufs=4,
                                   name=f"hsb{b}_{t}")
                    with nc.allow_low_precision(reason="bf16 hs"):
                        nc.gpsimd.tensor_copy(hb, st)
                    hsb.append(hb)
                return hsb

            def emit_hsT_transposes(b, thalf, hsb):
                ht = hsT[b % 2]
                for c in range(CT):
                    ps_tr = psum.tile([P, NQ], BF16, tag="p_big", bufs=2,
                                      name=f"ps_tr{b}_{thalf}_{c}")
                    for i in range(4):
                        nc.tensor.transpose(
                            ps_tr[:, i * P : (i + 1) * P],
                            hsb[i][:, c * P : (c + 1) * P],
                            identity_b,
                        )
                    nc.vector.tensor_copy(
                        ht[:, c, thalf * NQ : (thalf + 1) * NQ], ps_tr
                    )

            pending_hsb = emit_hs_prefetch(0, 0)
            # Q/K weight slices for pairs 0-1 staged upfront (after the hs
            # converts in the Pool queue); later pairs convert lazily inside
            # the C-window
            for hp in (0, 1):
                emit_qk_slice_load(hp)
                emit_qk_slice_load(CT + hp)
            for b in range(BL):
                ht = hsT[b % 2]
                emit_hsT_transposes(b, 0, pending_hsb)
                pending_hsb = emit_hs_prefetch(b, 1)
                emit_hsT_transposes(b, 1, pending_hsb)
                pending_hsb = None

                # ------- V-projection -------
                V = []
                for t in range(TT):
                    ps_v = psum.tile([P, D], F32, tag="p_av", bufs=2,
                                     name=f"ps_v{b}_{t}")
                    for c in range(CT):
                        for q in range(2):
                            sl = slice(q * NQ, (q + 1) * NQ)
                            nc.tensor.matmul(
                                ps_v[:, sl],
                                ht[:, c, t * P : (t + 1) * P],
                                w_all[:, c, 2 * D + q * NQ : 2 * D + (q + 1) * NQ],
                                start=(c == 0), stop=(c == CT - 1),
                            )
                    v_t = main.tile([P, H * (HD + 1)], BF16, tag=f"v{t}",
                                    name=f"V{b}_{t}")
                    v3 = v_t.rearrange("p (h e) -> p h e", h=H)
                    nc.vector.tensor_copy(v3[:, :, HD : HD + 1], ones_ph)
                    with nc.allow_low_precision(reason="bf16 V"):
                        nc.vector.tensor_tensor(
                            out=v3[:, :, 0:HD],
                            in0=ps_v.rearrange("p (h e) -> p h e", h=H),
                            in1=bcast_bv.rearrange("p (h e) -> p h e", h=H),
                            op=add,
                        )
                    V.append(v_t)

                # ------- C-window: per head pair -------
                attnT = [
                    main.tile([P, T], BF16, tag=f"attnT{g}", name=f"attnT{b}_{g}")
                    for g in range(CT)
                ]

                def emit_norm_recip(b, hp, ps_av, i):
                    h = 2 * hp + i
                    recip = pipe.tile([1, T], BF16, tag="recip", bufs=3,
                                      name=f"recip{b}_{h}")
                    with nc.allow_low_precision(reason="softmax recip bf16"):
                        nc.vector.reciprocal(recip, ps_av[i][HD : HD + 1, :])
                    return recip

                def emit_norm_apply(b, hp, ps_av, i, recip):
                    h = 2 * hp + i
                    g, r0 = hp, i * HD
                    bc = pipe.tile([HD, T], BF16, tag="bc", bufs=3,
                                   name=f"bc{b}_{h}")
                    nc.gpsimd.partition_broadcast(bc, recip)
                    with nc.allow_low_precision(reason="bf16 attnT"):
                        nc.vector.tensor_tensor(
                            out=attnT[g][r0 : r0 + HD, :],
                            in0=ps_av[i][0:HD, :], in1=bc, op=mult,
                        )

                prev_av = None
                for hp in range(H // 2):
                    if b == 0:
                        # stage pair hp+2's weights / wout chunks during this
                        # pair's window (Pool + DMA idle mid-C)
                        if hp + 2 < H // 2:
                            emit_qk_slice_load(hp + 2)
                            emit_qk_slice_load(CT + hp + 2)
                        if hp < 4:
                            emit_wout_load(2 * hp)
                            emit_wout_load(2 * hp + 1)
                    if prev_av is not None:
                        prev_recips = [
                            emit_norm_recip(b, hp - 1, prev_av, i)
                            for i in range(2)
                        ]
                    pair_dst = []
                    for idx, (which, jt) in enumerate(
                        (("q", hp), ("k", CT + hp))
                    ):
                        ps_qk = psum.tile([P, T], F32, tag="p_big", bufs=2,
                                          name=f"ps_qk{b}_{jt}")
                        for c in range(CT):
                            for q in range(2):
                                sl = slice(q * NQ, (q + 1) * NQ)
                                nc.tensor.matmul(
                                    ps_qk[:, sl],
                                    w_all[:, c, jt * P : (jt + 1) * P],
                                    ht[:, c, sl],
                                    start=(c == 0), stop=(c == CT - 1),
                                )
                        dst = main.tile([P, T], BF16, tag=f"{which}t{hp % 2}",
                                        name=f"{which.upper()}T{b}_{hp}")
                        with nc.allow_low_precision(reason="bf16 qk"):
                            nc.vector.tensor_scalar_add(
                                dst, ps_qk, bq[:, jt : jt + 1]
                            )
                        pair_dst.append(dst)
                        if prev_av is not None:
                            emit_norm_apply(b, hp - 1, prev_av, idx,
                                            prev_recips[idx])
                    prev_av = None
                    QTg, KTg = pair_dst

                    ps_av = [
                        psum.tile([HD + 1, T], F32, tag="p_av", bufs=2,
                                  name=f"ps_av{b}_{2 * hp + i}")
                        for i in range(2)
                    ]

                    def emit_qk_exp(kt):
                        expts = []
                        for i in range(2):
                            r0 = i * HD
                            ps_l = psum.tile([P, T], F32, tag="p_big", bufs=2,
                                             name=f"ps_l{b}_{hp}_{kt}_{i}")
                            for q in range(2):
                                sl = slice(q * NQ, (q + 1) * NQ)
                                nc.tensor.matmul(
                                    ps_l[:, sl],
                                    KTg[r0 : r0 + HD, kt * P : (kt + 1) * P],
                                    QTg[r0 : r0 + HD, sl],
                                    start=True, stop=True,
                                )
                            expt = pipe.tile([P, T], BF16, tag="exp", bufs=4,
                                             name=f"exp{b}_{hp}_{kt}_{i}")
                            with nc.allow_low_precision(reason="bf16 exp"):
                                nc.scalar.activation(expt, ps_l, Exp,
                                                     scale=float(SCALE))
                            expts.append(expt)
                        return expts

                    def emit_av(kt, expts):
                        for i in range(2):
                            h = 2 * hp + i
                            for q in range(2):
                                sl = slice(q * NQ, (q + 1) * NQ)
                                nc.tensor.matmul(
                                    ps_av[i][:, sl],
                                    V[kt][:, h * (HD + 1) : (h + 1) * (HD + 1)],
                                    expts[i][:, sl],
                                    start=(kt == 0), stop=(kt == TT - 1),
                                )

                    pend = emit_qk_exp(0)
                    for kt in range(1, TT):
                        nxt = emit_qk_exp(kt)
                        emit_av(kt - 1, pend)
                        pend = nxt
                    emit_av(TT - 1, pend)
                    prev_av = ps_av
                for i in range(2):
                    rc = emit_norm_recip(b, H // 2 - 1, prev_av, i)
                    emit_norm_apply(b, H // 2 - 1, prev_av, i, rc)

                # prefetch next item's first hs half: its DMA + Pool convert
                # run while the PE does the out-projection below
                if b + 1 < BL:
                    pending_hsb = emit_hs_prefetch(b + 1, 0)

                # ------- D: out projection -------
                for t in range(TT):
                    ps_o = psum.tile([P, D], F32, tag="p_big", bufs=2,
                                     name=f"ps_o{b}_{t}")
                    for dt in range(CT):
                        for e in range(2):
                            sl = slice(e * NQ, (e + 1) * NQ)
                            nc.tensor.matmul(
                                ps_o[:, sl],
                                attnT[dt][:, t * P : (t + 1) * P],
                                wout_sb[:, dt, sl],
                                start=(dt == 0), stop=(dt == CT - 1),
                            )
                    o_t = pipe.tile([P, D], F32, tag="o", name=f"o{b}_{t}")
                    nc.vector.tensor_tensor(
                        out=o_t, in0=ps_o, in1=bcast_bout, op=add
                    )
                    nc.gpsimd.dma_start(
                        out=out[b, t * P : (t + 1) * P, :], in_=o_t
                    )

    nc.compile()
    return nc


_nc_cache = None


def kernel(**inputs) -> np.ndarray:
    global _nc_cache, _last_results
    hs = np.ascontiguousarray(np.asarray(inputs["hidden_states"], dtype=np.float32))
    w_qkv = np.ascontiguousarray(np.asarray(inputs["w_qkv"], dtype=np.float32))
    b_qkv = np.ascontiguousarray(np.asarray(inputs["b_qkv"], dtype=np.float32))
    w_out = np.ascontiguousarray(np.asarray(inputs["w_out"], dtype=np.float32))
    b_out = np.ascontiguousarray(np.asarray(inputs["b_out"], dtype=np.float32))

    if _nc_cache is None:
        _nc_cache = build_program()
    nc = _nc_cache

    in_maps = [
        {
            "hidden_states": hs[c * BL : (c + 1) * BL],
            "w_qkv": w_qkv,
            "b_qkv": b_qkv,
            "w_out": w_out,
            "b_out": b_out,
        }
        for c in range(N_CORES)
    ]
    try:
        res = run_bass_kernel_spmd(
            nc,
            in_maps,
            list(range(N_CORES)),
            trace=bool(os.environ.get("BASS_TRACE")),
        )
    except ModuleNotFoundError:
        prev = os.environ.get("BASS_NEVER_TRACE")
        os.environ["BASS_NEVER_TRACE"] = "1"
        try:
            res = run_bass_kernel_spmd(nc, in_maps, list(range(N_CORES)))
        finally:
            if prev is None:
                os.environ.pop("BASS_NEVER_TRACE", None)
            else:
                os.environ["BASS_NEVER_TRACE"] = prev
    _last_results = res
    return np.concatenate([res.results[c]["out"] for c in range(N_CORES)], axis=0)
